# revision 21
# baseline (speedup 1.0000x reference)
"""Trainium2 Bass kernel for nn_AttentionBlock (GroupNorm + self/cross attention + residual).

Sharding: data-parallel over batch B=16 across 8 cores (2 batches per core).
No collectives needed.

Dtypes: heavy matmuls (QKV/scores/att@V/out-proj) run in bf16 (1 cyc/col on PE,
FWL weight loads); GroupNorm statistics, softmax accumulation (PSUM fp32), the
softmax denominators and the residual path stay fp32/f32r.  The attention
branch's bf16 rounding is diluted in the final output by the fp32 residual.

Layout strategy per batch (x_0 arrives channel-major [C, H*W] = x^T):
  - GroupNorm in [C(part), L(free)]; per-partition stats via bn_stats/bn_aggr;
    group aggregation + per-channel broadcast via tiny f32r matmuls with 0/1
    selector matrices; affine applied by ACT (per-partition scale/bias),
    producing xn in bf16.
  - Q^T, K^T (and cond K^T) channel-major: out[ch, tok] = w^T.T @ xn^T.
  - V (and cond V) token-major with a ones-column per head: [128, heads, 65].
  - scores^T[k, q] = K^T.T @ Q^T per 128-key chunk -> fp32 PSUM; exp on ScalarE
    (scale=1/8 fused; no max-subtraction: |scores/8| < ~6 so exp is safe).
  - att@V: out^T[d, q] + sums row = V_aug.T @ exp^T accumulated in fp32 PSUM.
  - softmax normalization: broadcast the sums row via a contract-1 matmul,
    reciprocal_approx_fast, then one multiply into bf16 att tiles.
  - out-projection channel-major; bias + residual fused in one DVE op.

Phases of the two batches are interleaved (A0 B0 A1 C0 B1 D0 C1 D1) so batch
1's PE-heavy projections overlap batch 0's ScalarE-heavy softmax.
"""

import sys

for _p in ("/opt/trn_rl_repo",):
    if _p not in sys.path:
        sys.path.insert(0, _p)

import numpy as np

B, C, H, W = 16, 512, 32, 32
L = H * W            # 1024 tokens
NH, HD = 8, 64       # heads, head dim
LC, CC = 77, 768     # cond tokens, cond channels
LCP = 80             # cond tokens padded (even moving-dim for PE)
NCORES = 8
BPC = B // NCORES    # batches per core = 2
CI = C // 128        # 4 channel chunks
CCI = CC // 128      # 6 cond-channel chunks
TI = L // 128        # 8 token chunks
NKC = TI + 1         # 9 key chunks (8 self + 1 cond)
SCALE = HD ** -0.5   # 0.125
INV_N = 1.0 / 64     # group stats = mean over the 64 partitions of per-partition stats
EPS = 1e-5

_CACHE = {}


def _build_program():
    import concourse.bass as bass
    import concourse.tile as tile
    from concourse import bacc, mybir

    F32 = mybir.dt.float32
    F32R = mybir.dt.float32r
    BF16 = mybir.dt.bfloat16
    AF = mybir.ActivationFunctionType
    ALU = mybir.AluOpType

    # Bacc (not plain Bass): its finalize() runs move_matmul_waits_to_ldweights /
    # generate_event_semaphores, required because fused-LDW matmuls can carry
    # only one HW sync wait.
    nc = bacc.Bacc(None)

    x0_d = nc.dram_tensor("x0", [BPC, C, L], F32R, kind="ExternalInput")
    condT_d = nc.dram_tensor("condT", [BPC, CC, LC], BF16, kind="ExternalInput")
    wqk_d = nc.dram_tensor("wqkT", [C, 2 * C], BF16, kind="ExternalInput")
    wv_d = nc.dram_tensor("wvT", [C, C], BF16, kind="ExternalInput")
    wkc_d = nc.dram_tensor("wkcT", [CC, C], BF16, kind="ExternalInput")
    wvc_d = nc.dram_tensor("wvcT", [CC, C], BF16, kind="ExternalInput")
    wout_d = nc.dram_tensor("woutT", [C, C], BF16, kind="ExternalInput")
    bqk_d = nc.dram_tensor("bqk", [128, 8], F32, kind="ExternalInput")
    bv_d = nc.dram_tensor("bv", [1, C], BF16, kind="ExternalInput")
    bkc_d = nc.dram_tensor("bkc", [128, 4], F32, kind="ExternalInput")
    bvc_d = nc.dram_tensor("bvc", [1, C], BF16, kind="ExternalInput")
    bout_d = nc.dram_tensor("bout", [128, 4], F32, kind="ExternalInput")
    gam_d = nc.dram_tensor("gam", [128, 4], F32, kind="ExternalInput")
    bet_d = nc.dram_tensor("bet", [128, 4], F32, kind="ExternalInput")
    selA_d = nc.dram_tensor("selA", [128, 32], F32R, kind="ExternalInput")
    selB_d = nc.dram_tensor("selB", [8, 512], F32R, kind="ExternalInput")
    ones_d = nc.dram_tensor("ones", [1, 128], BF16, kind="ExternalInput")
    onesr_d = nc.dram_tensor("onesr", [1, 128], F32R, kind="ExternalInput")
    onec_d = nc.dram_tensor("onec", [128, 1], BF16, kind="ExternalInput")
    y_d = nc.dram_tensor("y", [BPC, C, L], F32, kind="ExternalOutput")

    with tile.TileContext(nc) as tc:
        with (
            tc.tile_pool(name="const", bufs=1) as cpool,
            tc.tile_pool(name="x0p", bufs=2) as x0p,
            tc.tile_pool(name="gnp", bufs=2) as gnp,
            tc.tile_pool(name="xnp", bufs=2) as xnp,
            tc.tile_pool(name="qkp", bufs=16) as qkp,
            tc.tile_pool(name="kcp", bufs=8) as kcp,
            tc.tile_pool(name="vp", bufs=16) as vp,
            tc.tile_pool(name="vcp", bufs=2) as vcp,
            tc.tile_pool(name="condp", bufs=2) as condp,
            tc.tile_pool(name="ep", bufs=4) as ep,
            tc.tile_pool(name="attp", bufs=8) as attp,
            tc.tile_pool(name="sump", bufs=2) as sump,
            tc.tile_pool(name="bcsp", bufs=2) as bcsp,
            tc.tile_pool(name="resp", bufs=2) as resp,
            tc.tile_pool(name="outp", bufs=2) as outp,
            tc.tile_pool(name="psum", bufs=2, space="PSUM") as pp,
        ):
            st = {}  # per-batch tiles

            # ---------------- phase A: load x0 + GroupNorm ----------------
            def phase_a(b):
                x0_t = x0p.tile([128, CI, L], F32R, name=f"x0_{b}", tag="x0")
                x0v = x0_d[b].rearrange("(i p) l -> p i l", p=128)
                for i in range(CI):
                    nc.sync.dma_start(out=x0_t[:, i, :], in_=x0v[:, i, :])

                stats = gnp.tile([128, CI, 2], F32, name=f"st_{b}", tag="st")
                statsr = gnp.tile([128, CI, 2], F32R, name=f"str_{b}", tag="str")
                sq_t = gnp.tile([128, CI], F32, name=f"sq_{b}", tag="sq")
                for i in range(CI):
                    st6 = gnp.tile([128, 2, 6], F32, name=f"st6_{b}_{i}", tag="st6")
                    xv = x0_t[:, i, :].rearrange("p (s f) -> p s f", s=2)
                    for s in range(2):
                        nc.vector.bn_stats(out=st6[:, s, :], in_=xv[:, s, :])
                    nc.vector.bn_aggr(out=stats[:, i, :], in_=st6)
                    # per-partition E[x^2] = var + mean^2
                    nc.vector.tensor_mul(out=sq_t[:, i:i + 1], in0=stats[:, i, 0:1], in1=stats[:, i, 0:1])
                    nc.vector.tensor_add(out=stats[:, i, 1:2], in0=stats[:, i, 1:2], in1=sq_t[:, i:i + 1])
                    nc.vector.tensor_copy(out=statsr[:, i, :], in_=stats[:, i, :])

                gps = pp.tile([8, 2], F32, name=f"gps_{b}", tag="s")
                for i in range(CI):
                    nc.tensor.matmul(gps, selA_sb[:, 8 * i:8 * i + 8], statsr[:, i, :],
                                     start=(i == 0), stop=(i == CI - 1))
                mv = gnp.tile([8, 2], F32, name=f"mv_{b}", tag="mv")
                nc.scalar.mul(out=mv, in_=gps, mul=INV_N)
                m2 = gnp.tile([8, 1], F32, name=f"m2_{b}", tag="m2")
                nc.vector.tensor_mul(out=m2, in0=mv[:, 0:1], in1=mv[:, 0:1])
                var = gnp.tile([8, 1], F32, name=f"var_{b}", tag="var")
                nc.vector.tensor_sub(out=var, in0=mv[:, 1:2], in1=m2)
                std = gnp.tile([8, 1], F32, name=f"std_{b}", tag="std")
                nc.scalar.activation(out=std, in_=var, func=AF.Sqrt, bias=eps_sb[0:8], scale=1.0)
                mr = gnp.tile([8, 2], F32R, name=f"mr_{b}", tag="mr")
                nc.vector.tensor_copy(out=mr[:, 0:1], in_=mv[:, 0:1])
                with nc.allow_low_precision(reason="groupnorm rstd rounded to f32r"):
                    nc.vector.reciprocal(out=mr[:, 1:2], in_=std)

                ab_t = gnp.tile([128, CI, 2], F32, name=f"ab_{b}", tag="ab")
                tmp_t = gnp.tile([128, CI], F32, name=f"tmp_{b}", tag="tmp")
                for i in range(CI):
                    bps = pp.tile([128, 2], F32, name=f"bps_{b}_{i}", tag="s")
                    nc.tensor.matmul(bps, selB_sb[:, 128 * i:128 * i + 128], mr,
                                     start=True, stop=True)
                    # a = gamma * rstd ; bb = beta - mean * a
                    nc.vector.tensor_mul(out=ab_t[:, i, 0:1], in0=gam_sb[:, i:i + 1], in1=bps[:, 1:2])
                    nc.vector.tensor_mul(out=tmp_t[:, i:i + 1], in0=bps[:, 0:1], in1=ab_t[:, i, 0:1])
                    nc.vector.tensor_sub(out=ab_t[:, i, 1:2], in0=bet_sb[:, i:i + 1], in1=tmp_t[:, i:i + 1])

                xn_t = xnp.tile([128, CI, L], BF16, name=f"xn_{b}", tag="xn")
                for i in range(CI):
                    nc.scalar.activation(out=xn_t[:, i, :], in_=x0_t[:, i, :], func=AF.Identity,
                                         bias=ab_t[:, i, 1:2], scale=ab_t[:, i, 0:1])
                st[b] = {"xn": xn_t}

            # ---------------- phase B: projections ----------------
            def phase_b(b):
                xn_t = st[b]["xn"]
                qk_t = []
                for o in range(8):  # q rows 0-511, k rows 512-1023
                    ps = pp.tile([128, L], F32, name=f"qkps_{b}_{o}", tag="s")
                    for i in range(CI):
                        for t2 in range(2):
                            nc.tensor.matmul(
                                ps[:, 512 * t2:512 * t2 + 512],
                                wqk_sb[:, i, 128 * o:128 * o + 128],
                                xn_t[:, i, 512 * t2:512 * t2 + 512],
                                start=(i == 0), stop=(i == CI - 1),
                            )
                    qt = qkp.tile([128, L], BF16, name=f"qk_{b}_{o}", tag="qk")
                    nc.vector.tensor_scalar_add(out=qt, in0=ps, scalar1=bqk_sb[:, o:o + 1])
                    qk_t.append(qt)

                v_t = []
                for t in range(TI):
                    ps = pp.tile([128, C], F32, name=f"vps_{b}_{t}", tag="s")
                    for i in range(CI):
                        nc.tensor.matmul(ps, xn_t[:, i, 128 * t:128 * t + 128],
                                         wv_sb[:, i, :], start=(i == 0), stop=False)
                    nc.tensor.matmul(ps, ones_sb[:, 0:128], bv_sb, start=False, stop=True)
                    vt = vp.tile([128, NH, HD + 1], BF16, name=f"v_{b}_{t}", tag="v")
                    nc.vector.tensor_copy(out=vt[:, :, 0:HD], in_=ps.rearrange("p (h d) -> p h d", h=NH))
                    nc.vector.tensor_copy(out=vt[:, :, HD:HD + 1], in_=onec_sb.to_broadcast((128, NH, 1)))
                    v_t.append(vt)

                cond_t = condp.tile([128, CCI, LCP], BF16, name=f"cond_{b}", tag="cond")
                nc.vector.memset(cond_t.bitcast(mybir.dt.uint16)[:, :, LC:LCP], 0)
                nc.sync.dma_start(out=cond_t[:, :, 0:LC], in_=condT_d[b].rearrange("(i p) l -> p i l", p=128))
                kc_t = []
                for o in range(4):
                    ps = pp.tile([128, LCP], F32, name=f"kcps_{b}_{o}", tag="s")
                    for i in range(CCI):
                        nc.tensor.matmul(ps, wkc_sb[:, i, 128 * o:128 * o + 128],
                                         cond_t[:, i, :], start=(i == 0), stop=(i == CCI - 1))
                    kt = kcp.tile([128, LC], BF16, name=f"kc_{b}_{o}", tag="kc")
                    nc.vector.tensor_scalar_add(out=kt, in0=ps[:, 0:LC], scalar1=bkc_sb[:, o:o + 1])
                    kc_t.append(kt)

                vps = pp.tile([LC, C], F32, name=f"vcps_{b}", tag="s")
                for i in range(CCI):
                    nc.tensor.matmul(vps, cond_t[:, i, 0:LC], wvc_sb[:, i, :],
                                     start=(i == 0), stop=False)
                nc.tensor.matmul(vps, ones_sb[:, 0:LC], bvc_sb, start=False, stop=True)
                vc_t = vcp.tile([LC, NH, HD + 1], BF16, name=f"vc_{b}", tag="vc")
                nc.vector.tensor_copy(out=vc_t[:, :, 0:HD], in_=vps.rearrange("p (h d) -> p h d", h=NH))
                nc.vector.tensor_copy(out=vc_t[:, :, HD:HD + 1], in_=onec_sb[0:LC].to_broadcast((LC, NH, 1)))
                st[b].update(qk=qk_t, v=v_t, kc=kc_t, vc=vc_t)

            # ---------------- phase C: attention ----------------
            def phase_c(b):
                qk_t, v_t, kc_t, vc_t = st[b]["qk"], st[b]["v"], st[b]["kc"], st[b]["vc"]
                att_t = [attp.tile([128, L], BF16, name=f"att_{b}_{j}", tag="att") for j in range(CI)]
                for h in range(NH):
                    ci, po = h // 2, 64 * (h % 2)
                    qT = qk_t[ci][po:po + 64, :]          # [64, L]
                    kT = qk_t[4 + ci][po:po + 64, :]      # [64, L]
                    kcT = kc_t[ci][po:po + 64, :]         # [64, LC]
                    av = pp.tile([HD + 1, L], F32, name=f"av_{b}_{h}", tag="av")
                    for kc in range(NKC):
                        kn = 128 if kc < TI else LC
                        sp = pp.tile([128, L], F32, name=f"sp_{b}_{h}_{kc}", tag="s")
                        lk = kT[:, 128 * kc:128 * kc + 128] if kc < TI else kcT
                        for t2 in range(2):
                            nc.tensor.matmul(sp[0:kn, 512 * t2:512 * t2 + 512], lk,
                                             qT[:, 512 * t2:512 * t2 + 512],
                                             start=True, stop=True)
                        et = ep.tile([128, L], BF16, name=f"e_{b}_{h}_{kc}", tag="e")
                        nc.scalar.activation(out=et[0:kn, :], in_=sp[0:kn, :], func=AF.Exp,
                                             bias=0.0, scale=SCALE)
                        vs = v_t[kc][:, h, :] if kc < TI else vc_t[:, h, :]  # [kn, 65]
                        for t2 in range(2):
                            nc.tensor.matmul(av[:, 512 * t2:512 * t2 + 512], vs,
                                             et[0:kn, 512 * t2:512 * t2 + 512],
                                             start=(kc == 0), stop=(kc == NKC - 1))
                    # softmax normalization: broadcast sums row, fast reciprocal, multiply
                    sums = sump.tile([1, L], F32R, name=f"sums_{b}_{h}", tag="sums")
                    nc.vector.tensor_copy(out=sums, in_=av[HD:HD + 1, :])
                    bc = pp.tile([64, L], F32, name=f"bc_{b}_{h}", tag="av")
                    for t2 in range(2):
                        nc.tensor.matmul(bc[:, 512 * t2:512 * t2 + 512], onesr_sb[:, 0:64],
                                         sums[:, 512 * t2:512 * t2 + 512], start=True, stop=True)
                    bcs = bcsp.tile([64, L], F32, name=f"bcs_{b}_{h}", tag="bcs")
                    nc.vector.reciprocal_approx_fast(out=bcs, in_=bc)
                    nc.vector.tensor_mul(out=att_t[ci][po:po + 64, :], in0=av[0:64, :], in1=bcs)
                st[b]["att"] = att_t

            # ---------------- phase D: out projection + residual ----------------
            def phase_d(b):
                att_t = st[b]["att"]
                for o in range(CI):
                    ps = pp.tile([128, L], F32, name=f"ops_{b}_{o}", tag="s")
                    for i in range(CI):
                        for t2 in range(2):
                            nc.tensor.matmul(
                                ps[:, 512 * t2:512 * t2 + 512],
                                wout_sb[:, i, 128 * o:128 * o + 128],
                                att_t[i][:, 512 * t2:512 * t2 + 512],
                                start=(i == 0), stop=(i == CI - 1),
                            )
                    res_t = resp.tile([128, L], F32R, name=f"res_{b}_{o}", tag="res")
                    nc.sync.dma_start(out=res_t, in_=x0_d[b, 128 * o:128 * o + 128, :])
                    ot = outp.tile([128, L], F32, name=f"out_{b}_{o}", tag="out")
                    nc.vector.scalar_tensor_tensor(out=ot, in0=ps, scalar=bout_sb[:, o:o + 1],
                                                   in1=res_t, op0=ALU.add, op1=ALU.add)
                    nc.sync.dma_start(out=y_d[b, 128 * o:128 * o + 128, :], in_=ot)

            # ---------------- emit: consts + interleaved batch phases ----------------
            # First the small constants (cheap DMAs), then phase A of batch 0 so
            # GroupNorm overlaps the big weight loads.
            bqk_sb = cpool.tile([128, 8], F32, name="bqk_sb", tag="bqk")
            nc.sync.dma_start(out=bqk_sb, in_=bqk_d[:, :])
            bkc_sb = cpool.tile([128, 4], F32, name="bkc_sb", tag="bkc")
            nc.sync.dma_start(out=bkc_sb, in_=bkc_d[:, :])
            bout_sb = cpool.tile([128, 4], F32, name="bout_sb", tag="bout")
            nc.sync.dma_start(out=bout_sb, in_=bout_d[:, :])
            gam_sb = cpool.tile([128, 4], F32, name="gam_sb", tag="gam")
            nc.sync.dma_start(out=gam_sb, in_=gam_d[:, :])
            bet_sb = cpool.tile([128, 4], F32, name="bet_sb", tag="bet")
            nc.sync.dma_start(out=bet_sb, in_=bet_d[:, :])
            selA_sb = cpool.tile([128, 32], F32R, name="selA_sb", tag="selA")
            nc.sync.dma_start(out=selA_sb, in_=selA_d[:, :])
            selB_sb = cpool.tile([8, 512], F32R, name="selB_sb", tag="selB")
            nc.sync.dma_start(out=selB_sb, in_=selB_d[:, :])
            ones_sb = cpool.tile([1, 128], BF16, name="ones_sb", tag="ones")
            nc.sync.dma_start(out=ones_sb, in_=ones_d[:, :])
            onesr_sb = cpool.tile([1, 128], F32R, name="onesr_sb", tag="onesr")
            nc.sync.dma_start(out=onesr_sb, in_=onesr_d[:, :])
            onec_sb = cpool.tile([128, 1], BF16, name="onec_sb", tag="onec")
            nc.sync.dma_start(out=onec_sb, in_=onec_d[:, :])
            eps_sb = cpool.tile([128, 1], F32, name="eps_sb", tag="eps")
            nc.vector.memset(eps_sb, EPS)

            phase_a(0)

            wqk_sb = cpool.tile([128, CI, 2 * C], BF16, name="wqk_sb", tag="wqk")
            nc.sync.dma_start(out=wqk_sb, in_=wqk_d[:, :].rearrange("(i p) o -> p i o", p=128))
            wv_sb = cpool.tile([128, CI, C], BF16, name="wv_sb", tag="wv")
            nc.sync.dma_start(out=wv_sb, in_=wv_d[:, :].rearrange("(i p) o -> p i o", p=128))
            bv_sb = cpool.tile([1, C], BF16, name="bv_sb", tag="bv")
            nc.sync.dma_start(out=bv_sb, in_=bv_d[:, :])
            wkc_sb = cpool.tile([128, CCI, C], BF16, name="wkc_sb", tag="wkc")
            nc.sync.dma_start(out=wkc_sb, in_=wkc_d[:, :].rearrange("(i p) o -> p i o", p=128))
            wvc_sb = cpool.tile([128, CCI, C], BF16, name="wvc_sb", tag="wvc")
            nc.sync.dma_start(out=wvc_sb, in_=wvc_d[:, :].rearrange("(i p) o -> p i o", p=128))
            bvc_sb = cpool.tile([1, C], BF16, name="bvc_sb", tag="bvc")
            nc.sync.dma_start(out=bvc_sb, in_=bvc_d[:, :])
            wout_sb = cpool.tile([128, CI, C], BF16, name="wout_sb", tag="wout")
            nc.sync.dma_start(out=wout_sb, in_=wout_d[:, :].rearrange("(i p) o -> p i o", p=128))

            phase_b(0)
            phase_a(1)
            phase_c(0)
            phase_b(1)
            phase_d(0)
            phase_c(1)
            phase_d(1)

    nc.finalize()
    return nc


def _make_consts():
    selA = np.zeros((128, 32), np.float32)   # lhsT for group-stat aggregation
    selB = np.zeros((8, 512), np.float32)    # lhsT for group-stat broadcast
    for i in range(4):
        for p in range(128):
            g = 2 * i + (1 if p >= 64 else 0)
            selA[p, 8 * i + g] = 1.0
            selB[g, 128 * i + p] = 1.0
    return selA, selB


def _marshal(x_0, cond, gn_gamma, gn_beta, qkv_w, qkv_b, cond_kv_w, cond_kv_b, out_w, out_b):
    import ml_dtypes
    bf16 = ml_dtypes.bfloat16
    f = lambda a: np.ascontiguousarray(np.asarray(a, dtype=np.float32))
    h = lambda a: np.ascontiguousarray(np.asarray(a, dtype=np.float32).astype(bf16))
    x0r = f(x_0).reshape(B, C, L)
    condT = h(np.asarray(cond, dtype=np.float32).transpose(0, 2, 1))  # [B, CC, LC]
    qkv_w = f(qkv_w); cond_kv_w = f(cond_kv_w)
    shared = {
        "wqkT": h(qkv_w[: 2 * C].T),          # [512, 1024]
        "wvT": h(qkv_w[2 * C:].T),            # [512, 512]
        "wkcT": h(cond_kv_w[:C].T),           # [768, 512]
        "wvcT": h(cond_kv_w[C:].T),           # [768, 512]
        "woutT": h(np.asarray(out_w).T),      # [512, 512]
        "bqk": f(np.asarray(qkv_b)[: 2 * C].reshape(8, 128).T),
        "bv": h(np.asarray(qkv_b)[2 * C:].reshape(1, C)),
        "bkc": f(np.asarray(cond_kv_b)[:C].reshape(4, 128).T),
        "bvc": h(np.asarray(cond_kv_b)[C:].reshape(1, C)),
        "bout": f(np.asarray(out_b).reshape(4, 128).T),
        "gam": f(np.asarray(gn_gamma).reshape(4, 128).T),
        "bet": f(np.asarray(gn_beta).reshape(4, 128).T),
        "ones": np.ones((1, 128), bf16),
        "onesr": np.ones((1, 128), np.float32),
        "onec": np.ones((128, 1), bf16),
    }
    selA, selB = _make_consts()
    shared["selA"] = selA
    shared["selB"] = selB
    in_maps = []
    for c in range(NCORES):
        m = dict(shared)
        m["x0"] = np.ascontiguousarray(x0r[BPC * c:BPC * (c + 1)])
        m["condT"] = np.ascontiguousarray(condT[BPC * c:BPC * (c + 1)])
        in_maps.append(m)
    return in_maps


def run(trace=False, **inputs):
    from concourse.bass_utils import run_bass_kernel_spmd

    if "nc" not in _CACHE:
        _CACHE["nc"] = _build_program()
    nc = _CACHE["nc"]
    in_maps = _marshal(**inputs)
    res = run_bass_kernel_spmd(nc, in_maps, list(range(NCORES)), trace=trace)
    y = np.concatenate([res.results[c]["y"] for c in range(NCORES)], axis=0)
    out = y.reshape(B, C, H, W).astype(np.float32, copy=False)
    return out, res


def kernel(**inputs):
    out, _ = run(trace=False, **inputs)
    return out


# revision 22
# speedup vs baseline: 1.1651x; 1.1651x over previous
"""Trainium2 Bass kernel for nn_AttentionBlock (GroupNorm + self/cross attention + residual).

Sharding: data-parallel over batch B=16 across 8 cores (2 batches per core).
No collectives needed.

Dtypes: heavy matmuls (QKV/scores/att@V/out-proj) run in bf16 (1 cyc/col on PE,
FWL weight loads); GroupNorm statistics, softmax accumulation (PSUM fp32), the
softmax denominators and the residual path stay fp32/f32r.  The attention
branch's bf16 rounding is diluted in the final output by the fp32 residual.

Layout strategy per batch (x_0 arrives channel-major [C, H*W] = x^T):
  - GroupNorm in [C(part), L(free)]; per-partition stats via bn_stats/bn_aggr;
    group aggregation + per-channel broadcast via tiny f32r matmuls with 0/1
    selector matrices; affine applied by ACT (per-partition scale/bias),
    producing xn in bf16.
  - Q^T, K^T (and cond K^T) channel-major: out[ch, tok] = w^T.T @ xn^T.
  - V (and cond V) token-major with a ones-column per head: [128, heads, 65].
  - scores^T[k, q] = K^T.T @ Q^T per 128-key chunk -> fp32 PSUM; exp on ScalarE
    (scale=1/8 fused; no max-subtraction: |scores/8| < ~6 so exp is safe).
  - att@V: out^T[d, q] + sums row = V_aug.T @ exp^T accumulated in fp32 PSUM.
  - softmax normalization: broadcast the sums row via a contract-1 matmul,
    reciprocal_approx_fast, then one multiply into bf16 att tiles.
  - out-projection channel-major; bias + residual fused in one DVE op.

Phases of the two batches are interleaved (A0 B0 A1 C0 B1 D0 C1 D1) so batch
1's PE-heavy projections overlap batch 0's ScalarE-heavy softmax.
"""

import sys

for _p in ("/opt/trn_rl_repo",):
    if _p not in sys.path:
        sys.path.insert(0, _p)

import numpy as np

B, C, H, W = 16, 512, 32, 32
L = H * W            # 1024 tokens
NH, HD = 8, 64       # heads, head dim
LC, CC = 77, 768     # cond tokens, cond channels
LCP = 80             # cond tokens padded (even moving-dim for PE)
NCORES = 8
BPC = B // NCORES    # batches per core = 2
CI = C // 128        # 4 channel chunks
CCI = CC // 128      # 6 cond-channel chunks
TI = L // 128        # 8 token chunks
NKC = TI + 1         # 9 key chunks (8 self + 1 cond)
SCALE = HD ** -0.5   # 0.125
INV_N = 1.0 / 64     # group stats = mean over the 64 partitions of per-partition stats
EPS = 1e-5

_CACHE = {}


def _build_program():
    import concourse.bass as bass
    import concourse.tile as tile
    from concourse import bacc, mybir

    F32 = mybir.dt.float32
    F32R = mybir.dt.float32r
    BF16 = mybir.dt.bfloat16
    AF = mybir.ActivationFunctionType
    ALU = mybir.AluOpType

    # Bacc (not plain Bass): its finalize() runs move_matmul_waits_to_ldweights /
    # generate_event_semaphores, required because fused-LDW matmuls can carry
    # only one HW sync wait.
    nc = bacc.Bacc(None)

    x0_d = nc.dram_tensor("x0", [BPC, C, L], F32R, kind="ExternalInput")
    condT_d = nc.dram_tensor("condT", [BPC, CC, LC], BF16, kind="ExternalInput")
    wqk_d = nc.dram_tensor("wqkT", [C, 2 * C], BF16, kind="ExternalInput")
    wv_d = nc.dram_tensor("wvT", [C, C], BF16, kind="ExternalInput")
    wkc_d = nc.dram_tensor("wkcT", [CC, C], BF16, kind="ExternalInput")
    wvc_d = nc.dram_tensor("wvcT", [CC, C], BF16, kind="ExternalInput")
    wout_d = nc.dram_tensor("woutT", [C, C], BF16, kind="ExternalInput")
    bqk_d = nc.dram_tensor("bqk", [128, 8], F32, kind="ExternalInput")
    bv_d = nc.dram_tensor("bv", [1, C], BF16, kind="ExternalInput")
    bkc_d = nc.dram_tensor("bkc", [128, 4], F32, kind="ExternalInput")
    bvc_d = nc.dram_tensor("bvc", [1, C], BF16, kind="ExternalInput")
    bout_d = nc.dram_tensor("bout", [128, 4], F32, kind="ExternalInput")
    gam_d = nc.dram_tensor("gam", [128, 4], F32, kind="ExternalInput")
    bet_d = nc.dram_tensor("bet", [128, 4], F32, kind="ExternalInput")
    selA_d = nc.dram_tensor("selA", [128, 32], F32R, kind="ExternalInput")
    selB_d = nc.dram_tensor("selB", [8, 512], F32R, kind="ExternalInput")
    ones_d = nc.dram_tensor("ones", [1, 128], BF16, kind="ExternalInput")
    onesr_d = nc.dram_tensor("onesr", [1, 128], F32R, kind="ExternalInput")
    onec_d = nc.dram_tensor("onec", [128, 1], BF16, kind="ExternalInput")
    y_d = nc.dram_tensor("y", [BPC, C, L], F32, kind="ExternalOutput")

    with tile.TileContext(nc) as tc:
        with (
            tc.tile_pool(name="const", bufs=1) as cpool,
            tc.tile_pool(name="x0p", bufs=2) as x0p,
            tc.tile_pool(name="gnp", bufs=2) as gnp,
            tc.tile_pool(name="xnp", bufs=2) as xnp,
            tc.tile_pool(name="qkp", bufs=16) as qkp,
            tc.tile_pool(name="kcp", bufs=8) as kcp,
            tc.tile_pool(name="vp", bufs=16) as vp,
            tc.tile_pool(name="vcp", bufs=2) as vcp,
            tc.tile_pool(name="condp", bufs=2) as condp,
            tc.tile_pool(name="ep", bufs=3) as ep,
            tc.tile_pool(name="attp", bufs=8) as attp,
            tc.tile_pool(name="sump", bufs=2) as sump,
            tc.tile_pool(name="bcsp", bufs=2) as bcsp,
            tc.tile_pool(name="resp", bufs=2) as resp,
            tc.tile_pool(name="outp", bufs=2) as outp,
            tc.tile_pool(name="psum", bufs=2, space="PSUM") as pp,
        ):
            st = {}  # per-batch tiles

            # ---------------- phase A: load x0 + GroupNorm ----------------
            def phase_a(b):
                x0_t = x0p.tile([128, CI, L], F32R, name=f"x0_{b}", tag="x0")
                nc.sync.dma_start(out=x0_t, in_=x0_d[b].rearrange("(i p) l -> p i l", p=128))

                stats = gnp.tile([128, CI, 2], F32, name=f"st_{b}", tag="st")
                statsr = gnp.tile([128, CI, 2], F32R, name=f"str_{b}", tag="str")
                sq_t = gnp.tile([128, CI], F32, name=f"sq_{b}", tag="sq")
                for i in range(CI):
                    st6 = gnp.tile([128, 2, 6], F32, name=f"st6_{b}_{i}", tag="st6")
                    xv = x0_t[:, i, :].rearrange("p (s f) -> p s f", s=2)
                    for s in range(2):
                        nc.vector.bn_stats(out=st6[:, s, :], in_=xv[:, s, :])
                    nc.vector.bn_aggr(out=stats[:, i, :], in_=st6)
                    # per-partition E[x^2] = var + mean^2
                    nc.vector.tensor_mul(out=sq_t[:, i:i + 1], in0=stats[:, i, 0:1], in1=stats[:, i, 0:1])
                    nc.vector.tensor_add(out=stats[:, i, 1:2], in0=stats[:, i, 1:2], in1=sq_t[:, i:i + 1])
                    nc.vector.tensor_copy(out=statsr[:, i, :], in_=stats[:, i, :])

                gps = pp.tile([8, 2], F32, name=f"gps_{b}", tag="s")
                for i in range(CI):
                    nc.tensor.matmul(gps, selA_sb[:, 8 * i:8 * i + 8], statsr[:, i, :],
                                     start=(i == 0), stop=(i == CI - 1))
                mv = gnp.tile([8, 2], F32, name=f"mv_{b}", tag="mv")
                nc.scalar.mul(out=mv, in_=gps, mul=INV_N)
                m2 = gnp.tile([8, 1], F32, name=f"m2_{b}", tag="m2")
                nc.vector.tensor_mul(out=m2, in0=mv[:, 0:1], in1=mv[:, 0:1])
                var = gnp.tile([8, 1], F32, name=f"var_{b}", tag="var")
                nc.vector.tensor_sub(out=var, in0=mv[:, 1:2], in1=m2)
                std = gnp.tile([8, 1], F32, name=f"std_{b}", tag="std")
                nc.scalar.activation(out=std, in_=var, func=AF.Sqrt, bias=eps_sb[0:8], scale=1.0)
                mr = gnp.tile([8, 2], F32R, name=f"mr_{b}", tag="mr")
                nc.vector.tensor_copy(out=mr[:, 0:1], in_=mv[:, 0:1])
                with nc.allow_low_precision(reason="groupnorm rstd rounded to f32r"):
                    nc.vector.reciprocal(out=mr[:, 1:2], in_=std)

                ab_t = gnp.tile([128, CI, 2], F32, name=f"ab_{b}", tag="ab")
                tmp_t = gnp.tile([128, CI], F32, name=f"tmp_{b}", tag="tmp")
                for i in range(CI):
                    bps = pp.tile([128, 2], F32, name=f"bps_{b}_{i}", tag="s")
                    nc.tensor.matmul(bps, selB_sb[:, 128 * i:128 * i + 128], mr,
                                     start=True, stop=True)
                    # a = gamma * rstd ; bb = beta - mean * a
                    nc.vector.tensor_mul(out=ab_t[:, i, 0:1], in0=gam_sb[:, i:i + 1], in1=bps[:, 1:2])
                    nc.vector.tensor_mul(out=tmp_t[:, i:i + 1], in0=bps[:, 0:1], in1=ab_t[:, i, 0:1])
                    nc.vector.tensor_sub(out=ab_t[:, i, 1:2], in0=bet_sb[:, i:i + 1], in1=tmp_t[:, i:i + 1])

                xn_t = xnp.tile([128, CI, L], BF16, name=f"xn_{b}", tag="xn")
                for i in range(CI):
                    nc.scalar.activation(out=xn_t[:, i, :], in_=x0_t[:, i, :], func=AF.Identity,
                                         bias=ab_t[:, i, 1:2], scale=ab_t[:, i, 0:1])
                st[b] = {"xn": xn_t}

            # ---------------- phase B: projections ----------------
            def phase_b(b):
                xn_t = st[b]["xn"]
                qk_t = []
                for o in range(8):  # q rows 0-511, k rows 512-1023
                    ps = pp.tile([128, L], F32, name=f"qkps_{b}_{o}", tag="s")
                    for i in range(CI):
                        for t2 in range(2):
                            nc.tensor.matmul(
                                ps[:, 512 * t2:512 * t2 + 512],
                                wqk_sb[:, i, 128 * o:128 * o + 128],
                                xn_t[:, i, 512 * t2:512 * t2 + 512],
                                start=(i == 0), stop=(i == CI - 1),
                            )
                    qt = qkp.tile([128, L], BF16, name=f"qk_{b}_{o}", tag="qk")
                    nc.vector.tensor_scalar_add(out=qt, in0=ps, scalar1=bqk_sb[:, o:o + 1])
                    qk_t.append(qt)

                v_t = []
                for t in range(TI):
                    ps = pp.tile([128, C], F32, name=f"vps_{b}_{t}", tag="s")
                    for i in range(CI):
                        nc.tensor.matmul(ps, xn_t[:, i, 128 * t:128 * t + 128],
                                         wv_sb[:, i, :], start=(i == 0), stop=False)
                    nc.tensor.matmul(ps, ones_sb[:, 0:128], bv_sb, start=False, stop=True)
                    vt = vp.tile([128, NH, HD + 1], BF16, name=f"v_{b}_{t}", tag="v")
                    nc.vector.tensor_copy(out=vt[:, :, 0:HD], in_=ps.rearrange("p (h d) -> p h d", h=NH))
                    nc.vector.tensor_copy(out=vt[:, :, HD:HD + 1], in_=onec_sb.to_broadcast((128, NH, 1)))
                    v_t.append(vt)

                cond_t = condp.tile([128, CCI, LCP], BF16, name=f"cond_{b}", tag="cond")
                nc.vector.memset(cond_t.bitcast(mybir.dt.uint16)[:, :, LC:LCP], 0)
                nc.sync.dma_start(out=cond_t[:, :, 0:LC], in_=condT_d[b].rearrange("(i p) l -> p i l", p=128))
                kc_t = []
                for o in range(4):
                    ps = pp.tile([128, LCP], F32, name=f"kcps_{b}_{o}", tag="s")
                    for i in range(CCI):
                        nc.tensor.matmul(ps, wkc_sb[:, i, 128 * o:128 * o + 128],
                                         cond_t[:, i, :], start=(i == 0), stop=(i == CCI - 1))
                    kt = kcp.tile([128, LC], BF16, name=f"kc_{b}_{o}", tag="kc")
                    nc.vector.tensor_scalar_add(out=kt, in0=ps[:, 0:LC], scalar1=bkc_sb[:, o:o + 1])
                    kc_t.append(kt)

                vps = pp.tile([LC, C], F32, name=f"vcps_{b}", tag="s")
                for i in range(CCI):
                    nc.tensor.matmul(vps, cond_t[:, i, 0:LC], wvc_sb[:, i, :],
                                     start=(i == 0), stop=False)
                nc.tensor.matmul(vps, ones_sb[:, 0:LC], bvc_sb, start=False, stop=True)
                vc_t = vcp.tile([LC, NH, HD + 1], BF16, name=f"vc_{b}", tag="vc")
                nc.vector.tensor_copy(out=vc_t[:, :, 0:HD], in_=vps.rearrange("p (h d) -> p h d", h=NH))
                nc.vector.tensor_copy(out=vc_t[:, :, HD:HD + 1], in_=onec_sb[0:LC].to_broadcast((LC, NH, 1)))
                st[b].update(qk=qk_t, v=v_t, kc=kc_t, vc=vc_t)

            # ---------------- phase C: attention ----------------
            def phase_c(b):
                qk_t, v_t, kc_t, vc_t = st[b]["qk"], st[b]["v"], st[b]["kc"], st[b]["vc"]
                att_t = [attp.tile([128, L], BF16, name=f"att_{b}_{j}", tag="att") for j in range(CI)]
                for h in range(NH):
                    ci, po = h // 2, 64 * (h % 2)
                    qT = qk_t[ci][po:po + 64, :]          # [64, L]
                    kT = qk_t[4 + ci][po:po + 64, :]      # [64, L]
                    kcT = kc_t[ci][po:po + 64, :]         # [64, LC]
                    av = pp.tile([HD + 1, L], F32, name=f"av_{b}_{h}", tag="av")
                    for kc in range(NKC):
                        kn = 128 if kc < TI else LC
                        sp = pp.tile([128, L], F32, name=f"sp_{b}_{h}_{kc}", tag="s")
                        lk = kT[:, 128 * kc:128 * kc + 128] if kc < TI else kcT
                        for t2 in range(2):
                            nc.tensor.matmul(sp[0:kn, 512 * t2:512 * t2 + 512], lk,
                                             qT[:, 512 * t2:512 * t2 + 512],
                                             start=True, stop=True)
                        et = ep.tile([128, L], BF16, name=f"e_{b}_{h}_{kc}", tag="e")
                        nc.scalar.activation(out=et[0:kn, :], in_=sp[0:kn, :], func=AF.Exp,
                                             bias=0.0, scale=SCALE)
                        vs = v_t[kc][:, h, :] if kc < TI else vc_t[:, h, :]  # [kn, 65]
                        for t2 in range(2):
                            nc.tensor.matmul(av[:, 512 * t2:512 * t2 + 512], vs,
                                             et[0:kn, 512 * t2:512 * t2 + 512],
                                             start=(kc == 0), stop=(kc == NKC - 1))
                    # softmax normalization: broadcast sums row, fast reciprocal, multiply
                    sums = sump.tile([1, L], F32R, name=f"sums_{b}_{h}", tag="sums")
                    nc.vector.tensor_copy(out=sums, in_=av[HD:HD + 1, :])
                    bc = pp.tile([64, L], F32, name=f"bc_{b}_{h}", tag="av")
                    for t2 in range(2):
                        nc.tensor.matmul(bc[:, 512 * t2:512 * t2 + 512], onesr_sb[:, 0:64],
                                         sums[:, 512 * t2:512 * t2 + 512], start=True, stop=True)
                    bcs = bcsp.tile([64, L], F32, name=f"bcs_{b}_{h}", tag="bcs")
                    nc.vector.reciprocal_approx_fast(out=bcs, in_=bc)
                    nc.vector.tensor_mul(out=att_t[ci][po:po + 64, :], in0=av[0:64, :], in1=bcs)
                st[b]["att"] = att_t

            # ---------------- phase D: out projection + residual ----------------
            def phase_d(b):
                att_t = st[b]["att"]
                for o in range(CI):
                    ps = pp.tile([128, L], F32, name=f"ops_{b}_{o}", tag="s")
                    for i in range(CI):
                        for t2 in range(2):
                            nc.tensor.matmul(
                                ps[:, 512 * t2:512 * t2 + 512],
                                wout_sb[:, i, 128 * o:128 * o + 128],
                                att_t[i][:, 512 * t2:512 * t2 + 512],
                                start=(i == 0), stop=(i == CI - 1),
                            )
                    res_t = resp.tile([128, L], F32R, name=f"res_{b}_{o}", tag="res")
                    nc.sync.dma_start(out=res_t, in_=x0_d[b, 128 * o:128 * o + 128, :])
                    ot = outp.tile([128, L], F32, name=f"out_{b}_{o}", tag="out")
                    nc.vector.scalar_tensor_tensor(out=ot, in0=ps, scalar=bout_sb[:, o:o + 1],
                                                   in1=res_t, op0=ALU.add, op1=ALU.add)
                    nc.sync.dma_start(out=y_d[b, 128 * o:128 * o + 128, :], in_=ot)

            # ---------------- emit: consts + interleaved batch phases ----------------
            # First the small constants (cheap DMAs), then phase A of batch 0 so
            # GroupNorm overlaps the big weight loads.
            bqk_sb = cpool.tile([128, 8], F32, name="bqk_sb", tag="bqk")
            nc.sync.dma_start(out=bqk_sb, in_=bqk_d[:, :])
            bkc_sb = cpool.tile([128, 4], F32, name="bkc_sb", tag="bkc")
            nc.sync.dma_start(out=bkc_sb, in_=bkc_d[:, :])
            bout_sb = cpool.tile([128, 4], F32, name="bout_sb", tag="bout")
            nc.sync.dma_start(out=bout_sb, in_=bout_d[:, :])
            gam_sb = cpool.tile([128, 4], F32, name="gam_sb", tag="gam")
            nc.sync.dma_start(out=gam_sb, in_=gam_d[:, :])
            bet_sb = cpool.tile([128, 4], F32, name="bet_sb", tag="bet")
            nc.sync.dma_start(out=bet_sb, in_=bet_d[:, :])
            selA_sb = cpool.tile([128, 32], F32R, name="selA_sb", tag="selA")
            nc.sync.dma_start(out=selA_sb, in_=selA_d[:, :])
            selB_sb = cpool.tile([8, 512], F32R, name="selB_sb", tag="selB")
            nc.sync.dma_start(out=selB_sb, in_=selB_d[:, :])
            ones_sb = cpool.tile([1, 128], BF16, name="ones_sb", tag="ones")
            nc.sync.dma_start(out=ones_sb, in_=ones_d[:, :])
            onesr_sb = cpool.tile([1, 128], F32R, name="onesr_sb", tag="onesr")
            nc.sync.dma_start(out=onesr_sb, in_=onesr_d[:, :])
            onec_sb = cpool.tile([128, 1], BF16, name="onec_sb", tag="onec")
            nc.sync.dma_start(out=onec_sb, in_=onec_d[:, :])
            eps_sb = cpool.tile([128, 1], F32, name="eps_sb", tag="eps")
            nc.vector.memset(eps_sb, EPS)

            wqk_sb = cpool.tile([128, CI, 2 * C], BF16, name="wqk_sb", tag="wqk")
            nc.sync.dma_start(out=wqk_sb, in_=wqk_d[:, :].rearrange("(i p) o -> p i o", p=128))
            wv_sb = cpool.tile([128, CI, C], BF16, name="wv_sb", tag="wv")
            nc.sync.dma_start(out=wv_sb, in_=wv_d[:, :].rearrange("(i p) o -> p i o", p=128))
            bv_sb = cpool.tile([1, C], BF16, name="bv_sb", tag="bv")
            nc.sync.dma_start(out=bv_sb, in_=bv_d[:, :])
            wkc_sb = cpool.tile([128, CCI, C], BF16, name="wkc_sb", tag="wkc")
            nc.sync.dma_start(out=wkc_sb, in_=wkc_d[:, :].rearrange("(i p) o -> p i o", p=128))
            wvc_sb = cpool.tile([128, CCI, C], BF16, name="wvc_sb", tag="wvc")
            nc.sync.dma_start(out=wvc_sb, in_=wvc_d[:, :].rearrange("(i p) o -> p i o", p=128))
            bvc_sb = cpool.tile([1, C], BF16, name="bvc_sb", tag="bvc")
            nc.sync.dma_start(out=bvc_sb, in_=bvc_d[:, :])
            wout_sb = cpool.tile([128, CI, C], BF16, name="wout_sb", tag="wout")
            nc.sync.dma_start(out=wout_sb, in_=wout_d[:, :].rearrange("(i p) o -> p i o", p=128))

            phase_a(0)
            phase_b(0)
            phase_a(1)
            phase_c(0)
            phase_b(1)
            phase_d(0)
            phase_c(1)
            phase_d(1)

    nc.finalize()
    return nc


def _make_consts():
    selA = np.zeros((128, 32), np.float32)   # lhsT for group-stat aggregation
    selB = np.zeros((8, 512), np.float32)    # lhsT for group-stat broadcast
    for i in range(4):
        for p in range(128):
            g = 2 * i + (1 if p >= 64 else 0)
            selA[p, 8 * i + g] = 1.0
            selB[g, 128 * i + p] = 1.0
    return selA, selB


def _marshal(x_0, cond, gn_gamma, gn_beta, qkv_w, qkv_b, cond_kv_w, cond_kv_b, out_w, out_b):
    import ml_dtypes
    bf16 = ml_dtypes.bfloat16
    f = lambda a: np.ascontiguousarray(np.asarray(a, dtype=np.float32))
    h = lambda a: np.ascontiguousarray(np.asarray(a, dtype=np.float32).astype(bf16))
    x0r = f(x_0).reshape(B, C, L)
    condT = h(np.asarray(cond, dtype=np.float32).transpose(0, 2, 1))  # [B, CC, LC]
    qkv_w = f(qkv_w); cond_kv_w = f(cond_kv_w)
    shared = {
        "wqkT": h(qkv_w[: 2 * C].T),          # [512, 1024]
        "wvT": h(qkv_w[2 * C:].T),            # [512, 512]
        "wkcT": h(cond_kv_w[:C].T),           # [768, 512]
        "wvcT": h(cond_kv_w[C:].T),           # [768, 512]
        "woutT": h(np.asarray(out_w).T),      # [512, 512]
        "bqk": f(np.asarray(qkv_b)[: 2 * C].reshape(8, 128).T),
        "bv": h(np.asarray(qkv_b)[2 * C:].reshape(1, C)),
        "bkc": f(np.asarray(cond_kv_b)[:C].reshape(4, 128).T),
        "bvc": h(np.asarray(cond_kv_b)[C:].reshape(1, C)),
        "bout": f(np.asarray(out_b).reshape(4, 128).T),
        "gam": f(np.asarray(gn_gamma).reshape(4, 128).T),
        "bet": f(np.asarray(gn_beta).reshape(4, 128).T),
        "ones": np.ones((1, 128), bf16),
        "onesr": np.ones((1, 128), np.float32),
        "onec": np.ones((128, 1), bf16),
    }
    selA, selB = _make_consts()
    shared["selA"] = selA
    shared["selB"] = selB
    in_maps = []
    for c in range(NCORES):
        m = dict(shared)
        m["x0"] = np.ascontiguousarray(x0r[BPC * c:BPC * (c + 1)])
        m["condT"] = np.ascontiguousarray(condT[BPC * c:BPC * (c + 1)])
        in_maps.append(m)
    return in_maps


def run(trace=False, **inputs):
    from concourse.bass_utils import run_bass_kernel_spmd

    if "nc" not in _CACHE:
        _CACHE["nc"] = _build_program()
    nc = _CACHE["nc"]
    in_maps = _marshal(**inputs)
    res = run_bass_kernel_spmd(nc, in_maps, list(range(NCORES)), trace=trace)
    y = np.concatenate([res.results[c]["y"] for c in range(NCORES)], axis=0)
    out = y.reshape(B, C, H, W).astype(np.float32, copy=False)
    return out, res


def kernel(**inputs):
    out, _ = run(trace=False, **inputs)
    return out
